# revision 1
# baseline (speedup 1.0000x reference)
"""Trainium2 Bass kernel for nn_AstraloraLayer: y = x @ A.T (+ low-rank
surrogate path that cancels in the forward value).

Sharding: data-parallel over tokens. Each of the 8 cores computes
y[c] = x[c] @ A.T for its [2048, 4096] token shard; A = w.reshape(4096, 4096)
is replicated. No collectives.

Per-core kernel: Y.T[o, t] = sum_k A.T[k, o] * X.T[k, t], computed as
TensorE matmuls with A.T tiles stationary and X.T tiles moving, fp16
operands accumulated in fp32 PSUM. X.T is resident in SBUF; A.T streams.
Host transposes the Y.T output back.
"""

import sys

import numpy as np

if "/opt/trn_rl_repo" not in sys.path:
    sys.path.insert(0, "/opt/trn_rl_repo")

D = 4096          # d_inp == d_out
TOK = 2048        # tokens per core (8 * 2048 total)
N_CORES = 8
P = 128           # partitions
KH = D // P       # 32 k-tiles over the contraction dim
OBLK = 256        # output-channel block streamed per A.T DMA
NOB = D // OBLK   # 16 blocks
NT = TOK // 512   # 4 moving-dim (token) blocks of 512

_COMPILED = None


def _build():
    import concourse.mybir as mybir
    import concourse.tile as tile
    from concourse import bacc

    f16 = mybir.dt.float16
    f32 = mybir.dt.float32

    nc = bacc.Bacc("TRN2", target_bir_lowering=False)

    # xt: X.T in partition-major layout [p, kh, t] with k = kh*128 + p
    xt_ext = nc.declare_dram_parameter("xt", [P, KH, TOK], f16, isOutput=False)
    # at: A.T row-major [k, o]
    at_ext = nc.declare_dram_parameter("at", [D, D], f16, isOutput=False)
    # out: Y.T [o, t]
    out_ext = nc.declare_dram_parameter("out", [D, TOK], f32, isOutput=True)

    at_view = at_ext.rearrange("(kh p) o -> p kh o", p=P)

    with tile.TileContext(nc) as tc:
        with (
            tc.tile_pool(name="xt", bufs=1) as xt_pool,
            tc.tile_pool(name="at", bufs=2) as at_pool,
            tc.tile_pool(name="ps", bufs=2, space="PSUM") as ps_pool,
            tc.tile_pool(name="ys", bufs=2) as ys_pool,
        ):
            xt_tiles = []
            for kh in range(KH):
                t = xt_pool.tile([P, TOK], f16, tag=f"xt{kh}")
                nc.sync.dma_start(out=t[:], in_=xt_ext[:, kh, :])
                xt_tiles.append(t)

            for ob in range(NOB):
                at_t = at_pool.tile([P, KH, OBLK], f16, tag="at")
                nc.sync.dma_start(
                    out=at_t[:], in_=at_view[:, :, ob * OBLK : (ob + 1) * OBLK]
                )
                for osub in range(OBLK // P):
                    ot = ob * (OBLK // P) + osub
                    ps = ps_pool.tile([P, TOK], f32, tag="ps")
                    for kh in range(KH):
                        lhsT = at_t[:, kh, osub * P : (osub + 1) * P]
                        for tb in range(NT):
                            nc.tensor.matmul(
                                ps[:, tb * 512 : (tb + 1) * 512],
                                lhsT,
                                xt_tiles[kh][:, tb * 512 : (tb + 1) * 512],
                                start=(kh == 0),
                                stop=(kh == KH - 1),
                            )
                    ys = ys_pool.tile([P, TOK], f32, tag="ys")
                    nc.vector.tensor_copy(ys[:], ps[:])
                    nc.sync.dma_start(
                        out=out_ext[ot * P : (ot + 1) * P, :], in_=ys[:]
                    )

    nc.compile()
    return nc


def _get_compiled():
    global _COMPILED
    if _COMPILED is None:
        _COMPILED = _build()
    return _COMPILED


def kernel(x, w, U, S, V):
    from concourse.bass_utils import run_bass_kernel_spmd

    assert x.shape == (N_CORES, TOK, D)
    nc = _get_compiled()

    # A.T in fp16, shared by all cores
    at = np.ascontiguousarray(w.reshape(D, D).T).astype(np.float16)

    in_maps = []
    for c in range(N_CORES):
        # X.T -> [kh, p, t] -> [p, kh, t]
        xt = (
            np.ascontiguousarray(
                x[c].T.reshape(KH, P, TOK).transpose(1, 0, 2)
            ).astype(np.float16)
        )
        in_maps.append({"xt": xt, "at": at})

    res = run_bass_kernel_spmd(nc, in_maps, core_ids=list(range(N_CORES)))

    y = np.empty((N_CORES, TOK, D), dtype=np.float32)
    for c in range(N_CORES):
        y[c] = res.results[c]["out"].T
    return y
